# revision 21
# baseline (speedup 1.0000x reference)
"""Trainium2 Bass kernel for per-time-slice spatial self-attention + 1x1 conv.

Math per (b, t) slice (16 slices total):
    x      = x_in[b, :, t]          reshaped [C=64, P=2304]
    theta  = theta_w[t] @ x         [32, P]
    phi    = phi_w[t] @ x           [32, P]
    S      = theta.T @ phi / sqrt(32)          [P, P]
    A      = softmax(S, axis=-1)
    f      = x @ A.T  (f[c,p] = sum_q A[p,q] x[c,q])
    out    = out_w @ f + x

Sharding: the 16 slices are independent -> 2 slices per NeuronCore, no
collectives. Host precomputes the cheap channel projections (theta, phi,
v = out_w @ x) and packs layouts; the device runs the O(P^2) attention core:

  per p-chunk (4x512 + 256) accumulating in PSUM over 18 q-tiles of 128:
    scoresT[q, p] = sum_c phi[c, q] theta[c, p]   (PE, K=32, row-tiled:
                     q-tile qt runs on PE array strip 32*(qt%4) so 4
                     K=32 matmuls execute concurrently; phi is packed
                     per-strip and theta replicated on all 4 strips)
    E = exp(scoresT / sqrt(32))                   (ScalarE, PSUM -> SBUF)
    val[m, p] += vte[q, m]^T E[q, p]              (PE, m: 64 v-channels + ones
                                                   column -> softmax denom)
  then one DVE copy [65, w] PSUM->SBUF and a per-chunk DMA out.

  Hardware findings baked into this structure (probe.py bisections):
  - strip-tiled (tile_position) matmuls narrower than N=512 hard-hang
    the device, so p is zero-padded to 2560 = 5*512; exp/value/output
    run at the real 256 width on the tail chunk (scores-only pays pad).
  - concurrent strip matmuls accumulating into the SAME PSUM bank also
    hang; the K=128 value matmuls therefore stay full-array (full-array
    interleaved with strip matmuls is fine, incl. open accum groups).

The softmax normalization (divide rows 0..63 by the denominator row 64)
commutes with the final 1x1 conv, so it runs on the host together with
the residual add - the device never touches the 1-lane reciprocal path.

exp skips max-subtraction (scores ~ N(0,1), max |s| ~ 6; fp32-exact safe).
"""

import os
import sys

for _p in ("/opt/trn_rl_repo", "/root/.axon_site/_ro/trn_rl_repo"):
    if os.path.isdir(_p) and _p not in sys.path:
        sys.path.append(_p)

# The axon NTFF profiling hook (antenv.axon_hooks) is absent in this
# container; make sure run_bass_kernel_spmd never takes the trace path.
os.environ["BASS_NEVER_TRACE"] = "1"

import numpy as np
from contextlib import ExitStack

import concourse.bass as bass
import concourse.tile as tile
from concourse import bacc, mybir
from concourse.bass_utils import run_bass_kernel_spmd

B, C, T, H, W = 2, 64, 8, 48, 48
C2 = 32
P = H * W                      # 2304
N_CORES = 8
S_PER_CORE = (B * T) // N_CORES  # 2 slices per core
QT = P // 128                  # 18 q-tiles of 128
NSTRIP = int(os.environ.get("KERNEL_NSTRIP", "4"))  # PE row-tiling strips
NG = (QT + NSTRIP - 1) // NSTRIP  # phi column blocks for row tiling
GSZ = 3                        # q-tiles per exp group (3 PSUM banks)
P_PAD = 2560                   # p padded to 5*512: w=256 strip-tiled
                               # matmuls hard-hang the device (observed)
P_CHUNKS = [(o, 512) for o in range(0, P_PAD, 512)]
SCALE = 1.0 / np.sqrt(np.float32(C2))

F32 = mybir.dt.float32
# PE matmul streaming dtype for theta/phi/vte/E. bf16 streams 1 row/cycle
# on the PE with FWL weight loads (fastest, end-to-end max rel err ~8e-4:
# the softmax denominator rides the same rounded E, so most of the bf16
# error cancels in the normalization).
_MM_CFG = os.environ.get("KERNEL_MM_DT", "bf16")
MM_DT = {"bf16": mybir.dt.bfloat16, "f32r": mybir.dt.float32r,
         "f32": mybir.dt.float32}[_MM_CFG]
EXPF = mybir.ActivationFunctionType.Exp
# Hybrid exp: groups listed here run on VectorE as a Schraudolph bf16
# bit-trick (int16(s*S_A + S_B) bitcast to bf16) instead of ScalarE exp,
# so both engines chew the softmax concurrently. End-to-end rel err with
# the default split measured 2.4e-3 in fp64 modeling (gate 2e-2): the
# systematic exp bias cancels in the softmax ratio and the sawtooth
# averages out in the value contraction.
DVE_GS = {int(g) for g in
          os.environ.get("KERNEL_DVEG", "1,4").split(",") if g != ""}
S_A = float(np.log2(np.exp(1.0)) * 128.0 * SCALE)
S_B = float(127 * 128 - 6)
I16 = mybir.dt.int16
# diagnostics: comma list of stages to drop ("scores", "exp", "value")
_SKIP = set(filter(None, os.environ.get("KERNEL_SKIP", "").split(",")))

_CACHE = {}


def build_nc(repeat=1, skip=None):
    """Build the per-core Bass program (SPMD: same NEFF on all 8 cores).

    repeat > 1 re-runs the whole computation; used only for timing (the
    extra passes recompute and overwrite the same outputs).
    skip: stages to drop for diagnostic timing (default: env KERNEL_SKIP).
    """
    _skip = _SKIP if skip is None else set(skip)
    nc = bacc.Bacc("TRN2", target_bir_lowering=False, debug=False,
                   num_devices=N_CORES)
    # theta replicated on the 4 partition strips, phi packed per strip
    # (q-tile qt at strip 32*(qt%4), column block qt//4) for row tiling
    th_d = nc.dram_tensor("theta_rep", [S_PER_CORE, NSTRIP * C2, P_PAD],
                          MM_DT, kind="ExternalInput").ap()
    ph_d = nc.dram_tensor("phi_rep", [S_PER_CORE, NSTRIP * C2, NG * 128],
                          MM_DT, kind="ExternalInput").ap()
    vte_d = nc.dram_tensor("vte", [S_PER_CORE, 128, QT * (C + 1)], MM_DT,
                           kind="ExternalInput").ap()
    y_d = nc.dram_tensor("y", [S_PER_CORE, C + 1, P], F32,
                         kind="ExternalOutput").ap()

    with tile.TileContext(nc) as tc, ExitStack() as ctx:
        ins = ctx.enter_context(tc.tile_pool(name="ins", bufs=2))
        epool = ctx.enter_context(tc.tile_pool(name="epool", bufs=3))
        scp = ctx.enter_context(tc.tile_pool(name="scp", bufs=2, space="PSUM"))
        valp = ctx.enter_context(tc.tile_pool(name="valp", bufs=2,
                                              space="PSUM"))
        epi = ctx.enter_context(tc.tile_pool(name="epi", bufs=3))

        for s in [s for _ in range(repeat) for s in range(S_PER_CORE)]:
            th_sb = ins.tile([NSTRIP * C2, P_PAD], MM_DT, tag="th")
            nc.sync.dma_start(out=th_sb, in_=th_d[s])
            ph_sb = ins.tile([NSTRIP * C2, NG * 128], MM_DT, tag="ph")
            nc.sync.dma_start(out=ph_sb, in_=ph_d[s])
            vte_sb = ins.tile([128, QT, C + 1], MM_DT, tag="vte")
            nc.sync.dma_start(out=vte_sb, in_=vte_d[s].rearrange(
                "p (q m) -> p q m", q=QT))

            for (off, w) in P_CHUNKS:
                val = valp.tile([C + 1, w], F32, tag="val")
                wr = min(w, P - off)  # real (non-pad) width
                for g in range(QT // GSZ):
                    sc = scp.tile([128, GSZ, w], F32, tag="sc")
                    for j in range(GSZ):
                        qt = g * GSZ + j
                        g4, j4 = divmod(qt, NSTRIP)
                        sp = 32 * j4  # PE array strip for this q-tile
                        if "scores" in _skip:
                            # diag: tiny stand-in matmul keeps deps/alloc
                            nc.tensor.matmul(
                                out=sc[:, j, 0:16],
                                lhsT=ph_sb[0:C2, 0:128],
                                rhs=th_sb[0:C2, 0:16],
                                start=True, stop=True)
                            continue
                        # scoresT[q, p] = sum_c phi[c, q] * theta[c, p]
                        nc.tensor.matmul(
                            out=sc[:, j, :],
                            lhsT=ph_sb[sp:sp + C2,
                                       g4 * 128:(g4 + 1) * 128],
                            rhs=th_sb[sp:sp + C2, off:off + w],
                            start=True, stop=True,
                            tile_position=(sp, 0) if NSTRIP > 1 else None,
                        )
                    e_sb = epool.tile([128, GSZ, w], MM_DT, tag="E")
                    if "exp" in _skip:
                        # diag: tiny activation keeps deps/alloc
                        nc.scalar.activation(out=e_sb[:, :, 0:16],
                                             in_=sc[:, :, 0:16], func=EXPF,
                                             scale=float(SCALE))
                    elif g in DVE_GS:
                        # VectorE Schraudolph exp (see header comment)
                        nc.vector.tensor_scalar(
                            out=e_sb[:, :, 0:wr].bitcast(I16),
                            in0=sc[:, :, 0:wr],
                            scalar1=S_A, scalar2=S_B,
                            op0=mybir.AluOpType.mult,
                            op1=mybir.AluOpType.add)
                    elif wr < w:
                        # tail chunk: exp only the real columns
                        nc.scalar.activation(out=e_sb[:, :, 0:wr],
                                             in_=sc[:, :, 0:wr], func=EXPF,
                                             scale=float(SCALE))
                    else:
                        nc.scalar.activation(out=e_sb, in_=sc, func=EXPF,
                                             scale=float(SCALE))
                    if "value" in _skip:
                        continue
                    for j in range(GSZ):
                        qt = g * GSZ + j
                        # val[m, p] += sum_q vte[q, m] * E[q, p]
                        nc.tensor.matmul(
                            out=val[:, 0:wr],
                            lhsT=vte_sb[:, qt, :],
                            rhs=e_sb[:, j, 0:wr],
                            start=(qt == 0), stop=(qt == QT - 1),
                        )
                o_chunk = epi.tile([C + 1, 512], F32, tag="oc")
                nc.vector.tensor_copy(out=o_chunk[:, 0:wr],
                                      in_=val[:, 0:wr])
                nc.sync.dma_start(out=y_d[s][:, off:off + wr],
                                  in_=o_chunk[:, 0:wr])

    nc.compile()
    return nc


def _np_mm():
    if _MM_CFG == "bf16":
        import ml_dtypes
        return np.dtype(ml_dtypes.bfloat16)
    return np.dtype(np.float32)


def host_prep(x_in, theta_w, phi_w, out_w):
    """Per-core input maps: channel projections + device layouts (numpy)."""
    mmdt = _np_mm()
    x_in = np.ascontiguousarray(x_in, dtype=np.float32)
    theta_w = np.asarray(theta_w, dtype=np.float32)
    phi_w = np.asarray(phi_w, dtype=np.float32)
    out_w = np.asarray(out_w, dtype=np.float32)

    x = np.transpose(x_in, (0, 2, 1, 3, 4)).reshape(B, T, C, P)

    in_maps = []
    for k in range(N_CORES):
        th = np.zeros((S_PER_CORE, NSTRIP * C2, P_PAD), mmdt)
        ph = np.zeros((S_PER_CORE, NSTRIP * C2, NG * 128), mmdt)
        vte = np.empty((S_PER_CORE, 128, QT * (C + 1)), mmdt)
        for s in range(S_PER_CORE):
            g = k * S_PER_CORE + s
            b, t = divmod(g, T)
            xslice = x[b, t]                      # [C, P]
            theta = theta_w[t] @ xslice           # [32, P]
            phi = phi_w[t] @ xslice               # [32, P]
            # theta on all 4 strips; phi q-tile qt at strip 32*(qt%4),
            # column block qt//4 (row-tiled scores matmuls)
            th[s, :, :P] = np.tile(theta, (NSTRIP, 1))
            for qt in range(QT):
                g4, j4 = divmod(qt, NSTRIP)
                ph[s, 32 * j4:32 * (j4 + 1), 128 * g4:128 * (g4 + 1)] = \
                    phi[:, 128 * qt:128 * (qt + 1)]
            v = out_w @ xslice                    # [64, P]
            vt = np.empty((QT, 128, C + 1), mmdt)
            vt[:, :, :C] = v.T.reshape(QT, 128, C)
            vt[:, :, C] = 1.0                     # softmax-denominator column
            vte[s] = np.transpose(vt, (1, 0, 2)).reshape(128, QT * (C + 1))
        in_maps.append({"theta_rep": th, "phi_rep": ph, "vte": vte})
    return in_maps


def assemble(results, x_in):
    out = np.empty((B, C, T, H, W), np.float32)
    for k in range(N_CORES):
        y = results[k]["y"]  # [S_PER_CORE, C+1, P]
        for s in range(S_PER_CORE):
            g = k * S_PER_CORE + s
            b, t = divmod(g, T)
            # normalize by the softmax-denominator row, add the residual
            f = y[s][:C] / y[s][C:C + 1]
            out[b, :, t] = f.reshape(C, H, W) + x_in[b, :, t]
    return out


def kernel(x_in, theta_w, phi_w, out_w):
    if "nc" not in _CACHE:
        _CACHE["nc"] = build_nc()
    nc = _CACHE["nc"]
    in_maps = host_prep(x_in, theta_w, phi_w, out_w)
    res = run_bass_kernel_spmd(nc, in_maps, core_ids=list(range(N_CORES)))
    return assemble(res.results, np.asarray(x_in, dtype=np.float32))


# revision 22
# speedup vs baseline: 1.0615x; 1.0615x over previous
"""Trainium2 Bass kernel for per-time-slice spatial self-attention + 1x1 conv.

Math per (b, t) slice (16 slices total):
    x      = x_in[b, :, t]          reshaped [C=64, P=2304]
    theta  = theta_w[t] @ x         [32, P]
    phi    = phi_w[t] @ x           [32, P]
    S      = theta.T @ phi / sqrt(32)          [P, P]
    A      = softmax(S, axis=-1)
    f      = x @ A.T  (f[c,p] = sum_q A[p,q] x[c,q])
    out    = out_w @ f + x

Sharding: the 16 slices are independent -> 2 slices per NeuronCore, no
collectives. Host precomputes the cheap channel projections (theta, phi,
v = out_w @ x) and packs layouts; the device runs the O(P^2) attention core:

  per p-chunk (4x512 + 256) accumulating in PSUM over 18 q-tiles of 128:
    scoresT[q, p] = sum_c phi[c, q] theta[c, p]   (PE, K=32, row-tiled:
                     q-tile qt runs on PE array strip 32*(qt%4) so 4
                     K=32 matmuls execute concurrently; phi is packed
                     per-strip and theta replicated on all 4 strips)
    E = exp(scoresT / sqrt(32))                   (ScalarE, PSUM -> SBUF)
    val[m, p] += vte[q, m]^T E[q, p]              (PE, m: 64 v-channels + ones
                                                   column -> softmax denom)
  then one DVE copy [65, w] PSUM->SBUF and a per-chunk DMA out.

  Hardware findings baked into this structure (probe.py bisections):
  - strip-tiled (tile_position) matmuls narrower than N=512 hard-hang
    the device, so p is zero-padded to 2560 = 5*512; exp/value/output
    run at the real 256 width on the tail chunk (scores-only pays pad).
  - concurrent strip matmuls accumulating into the SAME PSUM bank also
    hang; the K=128 value matmuls therefore stay full-array (full-array
    interleaved with strip matmuls is fine, incl. open accum groups).

The softmax normalization (divide rows 0..63 by the denominator row 64)
commutes with the final 1x1 conv, so it runs on the host together with
the residual add - the device never touches the 1-lane reciprocal path.

exp skips max-subtraction (scores ~ N(0,1), max |s| ~ 6; fp32-exact safe).
"""

import os
import sys

for _p in ("/opt/trn_rl_repo", "/root/.axon_site/_ro/trn_rl_repo"):
    if os.path.isdir(_p) and _p not in sys.path:
        sys.path.append(_p)

# The axon NTFF profiling hook (antenv.axon_hooks) is absent in this
# container; make sure run_bass_kernel_spmd never takes the trace path.
os.environ["BASS_NEVER_TRACE"] = "1"

import numpy as np
from contextlib import ExitStack

import concourse.bass as bass
import concourse.tile as tile
from concourse import bacc, mybir
from concourse.bass_utils import run_bass_kernel_spmd

B, C, T, H, W = 2, 64, 8, 48, 48
C2 = 32
P = H * W                      # 2304
N_CORES = 8
S_PER_CORE = (B * T) // N_CORES  # 2 slices per core
QT = P // 128                  # 18 q-tiles of 128
NSTRIP = int(os.environ.get("KERNEL_NSTRIP", "4"))  # PE row-tiling strips
NG = (QT + NSTRIP - 1) // NSTRIP  # phi column blocks for row tiling
GSZ = 3                        # q-tiles per exp group (3 PSUM banks)
P_PAD = 2560                   # p padded to 5*512: w=256 strip-tiled
                               # matmuls hard-hang the device (observed)
P_CHUNKS = [(o, 512) for o in range(0, P_PAD, 512)]
SCALE = 1.0 / np.sqrt(np.float32(C2))

F32 = mybir.dt.float32
# PE matmul streaming dtype for theta/phi/vte/E. bf16 streams 1 row/cycle
# on the PE with FWL weight loads (fastest, end-to-end max rel err ~8e-4:
# the softmax denominator rides the same rounded E, so most of the bf16
# error cancels in the normalization).
_MM_CFG = os.environ.get("KERNEL_MM_DT", "bf16")
MM_DT = {"bf16": mybir.dt.bfloat16, "f32r": mybir.dt.float32r,
         "f32": mybir.dt.float32}[_MM_CFG]
EXPF = mybir.ActivationFunctionType.Exp
# Hybrid exp: groups listed here run on VectorE as a Schraudolph bf16
# bit-trick (int16(s*S_A + S_B) bitcast to bf16) instead of ScalarE exp,
# so both engines chew the softmax concurrently. End-to-end rel err with
# the default split measured 2.4e-3 in fp64 modeling (gate 2e-2): the
# systematic exp bias cancels in the softmax ratio and the sawtooth
# averages out in the value contraction.
DVE_GS = {int(g) for g in
          os.environ.get("KERNEL_DVEG", "1,4").split(",") if g != ""}
S_A = float(np.log2(np.exp(1.0)) * 128.0 * SCALE)
S_B = float(127 * 128 - 6)
I16 = mybir.dt.int16
# diagnostics: comma list of stages to drop ("scores", "exp", "value")
_SKIP = set(filter(None, os.environ.get("KERNEL_SKIP", "").split(",")))

_CACHE = {}


def build_nc(repeat=1, skip=None):
    """Build the per-core Bass program (SPMD: same NEFF on all 8 cores).

    repeat > 1 re-runs the whole computation; used only for timing (the
    extra passes recompute and overwrite the same outputs).
    skip: stages to drop for diagnostic timing (default: env KERNEL_SKIP).
    """
    _skip = _SKIP if skip is None else set(skip)
    nc = bacc.Bacc("TRN2", target_bir_lowering=False, debug=False,
                   num_devices=N_CORES)
    # theta replicated on the 4 partition strips, phi packed per strip
    # (q-tile qt at strip 32*(qt%4), column block qt//4) for row tiling
    th_d = nc.dram_tensor("theta_rep", [S_PER_CORE, NSTRIP * C2, P_PAD],
                          MM_DT, kind="ExternalInput").ap()
    ph_d = nc.dram_tensor("phi_rep", [S_PER_CORE, NSTRIP * C2, NG * 128],
                          MM_DT, kind="ExternalInput").ap()
    vte_d = nc.dram_tensor("vte", [S_PER_CORE, 128, QT * (C + 1)], MM_DT,
                           kind="ExternalInput").ap()
    y_d = nc.dram_tensor("y", [S_PER_CORE, C + 1, P], F32,
                         kind="ExternalOutput").ap()

    with tile.TileContext(nc) as tc, ExitStack() as ctx:
        ins = ctx.enter_context(tc.tile_pool(name="ins", bufs=2))
        epool = ctx.enter_context(tc.tile_pool(name="epool", bufs=4))
        scp = ctx.enter_context(tc.tile_pool(name="scp", bufs=2, space="PSUM"))
        valp = ctx.enter_context(tc.tile_pool(name="valp", bufs=2,
                                              space="PSUM"))
        epi = ctx.enter_context(tc.tile_pool(name="epi", bufs=3))

        for s in [s for _ in range(repeat) for s in range(S_PER_CORE)]:
            th_sb = ins.tile([NSTRIP * C2, P_PAD], MM_DT, tag="th")
            nc.sync.dma_start(out=th_sb, in_=th_d[s])
            ph_sb = ins.tile([NSTRIP * C2, NG * 128], MM_DT, tag="ph")
            nc.sync.dma_start(out=ph_sb, in_=ph_d[s])
            vte_sb = ins.tile([128, QT, C + 1], MM_DT, tag="vte")
            nc.sync.dma_start(out=vte_sb, in_=vte_d[s].rearrange(
                "p (q m) -> p q m", q=QT))

            for (off, w) in P_CHUNKS:
                val = valp.tile([C + 1, w], F32, tag="val")
                wr = min(w, P - off)  # real (non-pad) width
                for g in range(QT // GSZ):
                    sc = scp.tile([128, GSZ, w], F32, tag="sc")
                    for j in range(GSZ):
                        qt = g * GSZ + j
                        g4, j4 = divmod(qt, NSTRIP)
                        sp = 32 * j4  # PE array strip for this q-tile
                        if "scores" in _skip:
                            # diag: tiny stand-in matmul keeps deps/alloc
                            nc.tensor.matmul(
                                out=sc[:, j, 0:16],
                                lhsT=ph_sb[0:C2, 0:128],
                                rhs=th_sb[0:C2, 0:16],
                                start=True, stop=True)
                            continue
                        # scoresT[q, p] = sum_c phi[c, q] * theta[c, p]
                        nc.tensor.matmul(
                            out=sc[:, j, :],
                            lhsT=ph_sb[sp:sp + C2,
                                       g4 * 128:(g4 + 1) * 128],
                            rhs=th_sb[sp:sp + C2, off:off + w],
                            start=True, stop=True,
                            tile_position=(sp, 0) if NSTRIP > 1 else None,
                        )
                    e_sb = epool.tile([128, GSZ, w], MM_DT, tag="E")
                    if "exp" in _skip:
                        # diag: tiny activation keeps deps/alloc
                        nc.scalar.activation(out=e_sb[:, :, 0:16],
                                             in_=sc[:, :, 0:16], func=EXPF,
                                             scale=float(SCALE))
                    elif g in DVE_GS:
                        # VectorE Schraudolph exp (see header comment)
                        nc.vector.tensor_scalar(
                            out=e_sb[:, :, 0:wr].bitcast(I16),
                            in0=sc[:, :, 0:wr],
                            scalar1=S_A, scalar2=S_B,
                            op0=mybir.AluOpType.mult,
                            op1=mybir.AluOpType.add)
                    elif wr < w:
                        # tail chunk: exp only the real columns
                        nc.scalar.activation(out=e_sb[:, :, 0:wr],
                                             in_=sc[:, :, 0:wr], func=EXPF,
                                             scale=float(SCALE))
                    else:
                        nc.scalar.activation(out=e_sb, in_=sc, func=EXPF,
                                             scale=float(SCALE))
                    if "value" in _skip:
                        continue
                    for j in range(GSZ):
                        qt = g * GSZ + j
                        # val[m, p] += sum_q vte[q, m] * E[q, p]
                        nc.tensor.matmul(
                            out=val[:, 0:wr],
                            lhsT=vte_sb[:, qt, :],
                            rhs=e_sb[:, j, 0:wr],
                            start=(qt == 0), stop=(qt == QT - 1),
                        )
                o_chunk = epi.tile([C + 1, 512], F32, tag="oc")
                # ScalarE does the PSUM->SBUF copy: VectorE is busy with
                # its share of the exp groups, ScalarE has slack
                nc.scalar.copy(out=o_chunk[:, 0:wr], in_=val[:, 0:wr])
                nc.sync.dma_start(out=y_d[s][:, off:off + wr],
                                  in_=o_chunk[:, 0:wr])

    nc.compile()
    return nc


def _np_mm():
    if _MM_CFG == "bf16":
        import ml_dtypes
        return np.dtype(ml_dtypes.bfloat16)
    return np.dtype(np.float32)


def host_prep(x_in, theta_w, phi_w, out_w):
    """Per-core input maps: channel projections + device layouts (numpy)."""
    mmdt = _np_mm()
    x_in = np.ascontiguousarray(x_in, dtype=np.float32)
    theta_w = np.asarray(theta_w, dtype=np.float32)
    phi_w = np.asarray(phi_w, dtype=np.float32)
    out_w = np.asarray(out_w, dtype=np.float32)

    x = np.transpose(x_in, (0, 2, 1, 3, 4)).reshape(B, T, C, P)

    in_maps = []
    for k in range(N_CORES):
        th = np.zeros((S_PER_CORE, NSTRIP * C2, P_PAD), mmdt)
        ph = np.zeros((S_PER_CORE, NSTRIP * C2, NG * 128), mmdt)
        vte = np.empty((S_PER_CORE, 128, QT * (C + 1)), mmdt)
        for s in range(S_PER_CORE):
            g = k * S_PER_CORE + s
            b, t = divmod(g, T)
            xslice = x[b, t]                      # [C, P]
            theta = theta_w[t] @ xslice           # [32, P]
            phi = phi_w[t] @ xslice               # [32, P]
            # theta on all 4 strips; phi q-tile qt at strip 32*(qt%4),
            # column block qt//4 (row-tiled scores matmuls)
            th[s, :, :P] = np.tile(theta, (NSTRIP, 1))
            for qt in range(QT):
                g4, j4 = divmod(qt, NSTRIP)
                ph[s, 32 * j4:32 * (j4 + 1), 128 * g4:128 * (g4 + 1)] = \
                    phi[:, 128 * qt:128 * (qt + 1)]
            v = out_w @ xslice                    # [64, P]
            vt = np.empty((QT, 128, C + 1), mmdt)
            vt[:, :, :C] = v.T.reshape(QT, 128, C)
            vt[:, :, C] = 1.0                     # softmax-denominator column
            vte[s] = np.transpose(vt, (1, 0, 2)).reshape(128, QT * (C + 1))
        in_maps.append({"theta_rep": th, "phi_rep": ph, "vte": vte})
    return in_maps


def assemble(results, x_in):
    out = np.empty((B, C, T, H, W), np.float32)
    for k in range(N_CORES):
        y = results[k]["y"]  # [S_PER_CORE, C+1, P]
        for s in range(S_PER_CORE):
            g = k * S_PER_CORE + s
            b, t = divmod(g, T)
            # normalize by the softmax-denominator row, add the residual
            f = y[s][:C] / y[s][C:C + 1]
            out[b, :, t] = f.reshape(C, H, W) + x_in[b, :, t]
    return out


def kernel(x_in, theta_w, phi_w, out_w):
    if "nc" not in _CACHE:
        _CACHE["nc"] = build_nc()
    nc = _CACHE["nc"]
    in_maps = host_prep(x_in, theta_w, phi_w, out_w)
    res = run_bass_kernel_spmd(nc, in_maps, core_ids=list(range(N_CORES)))
    return assemble(res.results, np.asarray(x_in, dtype=np.float32))
